# revision 20
# baseline (speedup 1.0000x reference)
"""Trainium2 Bass kernel for the CoLa MoE-routing module.

Computation (reference semantics):
    att   = q @ Wk.T + bk                  [B, S]
    a     = softmax(top8_mask(att))        [B, S]  (8 nonzero per row)
    out   = sum_s a[:, s] * (x @ V0[s].T @ V1[s].T)   [B, O]

Sharding: expert-parallel over 8 NeuronCores (8 experts each). Each core
receives the full x/q (replicated) and its slice of V0/V1. The expert axis
is rotated per-core in Wk/bk so that every core's local experts are columns
0..7 of its own attention matrix (top-k/softmax are permutation invariant).
Per-core partial outputs are summed on the host.

Schedule notes (v4):
  - attention runs in SPLIT fp16: att = qh@Wh + qh@Wl + ql@Wh + bk with
    qh/ql (Wh/Wl) the fp16 hi/lo halves of q (Wk). Max att error ~9e-6,
    zero top-8 flips vs the fp32 reference -- and, critically, the PE HAM
    activity monitor counts fp16 matmuls (fp32 ones run cold and do not
    warm the clock gate).
  - inputs ship as two merged fp16 tensors so DMA pieces are big:
    qw = [Whl | bias | qh | ql] (one 1.3MB piece, scalar ring) and
    xv = [x | v0] (1MB pieces, sync ring), v1 in fp8e3m4. The two rings
    stream in parallel; pieces are issued up-front in need-order.
  - expert loop is software-pipelined (MM2 of expert j issues after MM1
    of expert j+PIPE); the h*a scale is an ACT PSUM->SBUF fp16 copy plus
    a 2x-mode fp16 DVE multiply.
  - fp16 warmup matmuls cover the initial DMA wait so the expert stream
    runs at the warm 2.4GHz clock.

Shapes are hardcoded for B=256, IN=1024, OUT=1024, SUB=128, S=64, k=8.
"""

import os

import numpy as np

import concourse.bass as bass
import concourse.bacc as bacc
import concourse.mybir as mybir
import concourse.tile as tile
from concourse import bass_utils
from concourse.masks import make_identity

B = 256
IN_F = 1024
OUT_F = 1024
SUB_F = 128
Q_F = 1024
N_SUB = 64
N_ACT = 8
N_CORES = 8
E_LOC = N_SUB // N_CORES  # 8 experts per core

P = 128
BT = B // P  # 2 batch tiles
KC = IN_F // P  # 8 contraction chunks
QC = Q_F // P

F32 = mybir.dt.float32
BF16 = mybir.dt.bfloat16
FP16 = mybir.dt.float16
F8E3 = mybir.dt.float8e3

# qw layout (fp16 columns): [Whl | bias | qh | ql]
QW_WHL = 0                       # QC chunks of [Wh_c | Wl_c], 128 cols each
QW_BIAS = QC * 128               # 1024: [bk (64) | zeros (64)]
QW_QH = QW_BIAS + P              # 1152
QW_QL = QW_QH + QC * B           # 3200
QW_COLS = QW_QL + QC * B         # 5248
# xv layout (fp16 columns): [x | v0]
XV_X = 0                         # KC chunks of B
XV_V0 = KC * B                   # 2048; then expert j at XV_V0 + j*1024
XV_COLS = XV_V0 + E_LOC * KC * SUB_F  # 10240

MOE_DTYPE = os.environ.get("MOE_DTYPE", "fp16")
# PE warmup matmuls to lift the HAM clock gate before real work arrives
WARMUP_MMS = int(os.environ.get("MOE_WARMUP", "20"))
# expert-loop software pipeline depth on the PE queue
PIPE = int(os.environ.get("MOE_PIPE", "3"))


def _build(mode: str):
    nc = bacc.Bacc("TRN2", target_bir_lowering=False, debug=False,
                   num_devices=N_CORES)

    # ---- DRAM I/O (per-core), partition-major so DMAs are contiguous ----
    qw_d = nc.dram_tensor("qw", [P, QW_COLS], FP16, kind="ExternalInput").ap()
    xv_d = nc.dram_tensor("xv", [P, XV_COLS], FP16, kind="ExternalInput").ap()
    v1t_d = nc.dram_tensor("v1t", [2, P, 4, OUT_F], F8E3,
                           kind="ExternalInput").ap()
    out_d = nc.dram_tensor("out_p", [B, OUT_F], FP16,
                           kind="ExternalOutput").ap()

    with tile.TileContext(nc) as tc:
        with (
            tc.tile_pool(name="singles", bufs=1) as singles,
            tc.tile_pool(name="weights", bufs=2) as wpool,
            tc.tile_pool(name="work", bufs=6) as work,
            tc.tile_pool(name="ps_misc", bufs=1, space="PSUM") as ps_misc,
            tc.tile_pool(name="ps_h", bufs=3, space="PSUM") as ps_h,
            tc.tile_pool(name="ps_out", bufs=1, space="PSUM") as ps_out,
        ):
            # ---- input DMAs, all issued up-front on the sync ring in
            # strict need-order (parallel rings just split the global
            # HBM bandwidth and delay the early tensors; measured) ----
            qw_sb = singles.tile([P, QW_COLS], FP16, tag="qw")
            nc.sync.dma_start(qw_sb[:, 0:QW_QL], qw_d[:, 0:QW_QL])
            xv_sb = singles.tile([P, XV_COLS], FP16, tag="xv")
            v1t_sb = [wpool.tile([P, 4, OUT_F], F8E3, tag="v1t",
                                 name=f"v1t{m}") for m in range(2)]
            c1 = XV_V0 + 2 * KC * SUB_F     # x + experts 0-1
            c2 = XV_V0 + 4 * KC * SUB_F     # experts 2-3
            # ql right behind qh: the full attention + softmax then
            # fills the PE wait for the first expert piece
            nc.sync.dma_start(qw_sb[:, QW_QL:QW_COLS],
                              qw_d[:, QW_QL:QW_COLS])
            nc.sync.dma_start(xv_sb[:, 0:c1], xv_d[:, 0:c1])
            nc.sync.dma_start(xv_sb[:, c1:c2], xv_d[:, c1:c2])
            nc.sync.dma_start(v1t_sb[0], v1t_d[0])
            nc.sync.dma_start(xv_sb[:, c2:XV_COLS], xv_d[:, c2:XV_COLS])
            nc.sync.dma_start(v1t_sb[1], v1t_d[1])

            # ---- constants ----
            ones_sb = singles.tile([1, P], FP16, tag="ones")
            nc.vector.memset(ones_sb, 1.0)
            # 16-bit warmup operand (HAM ignores fp32 matmuls)
            wtile_sb = singles.tile([P, 512], FP16, tag="wtile")
            nc.vector.memset(wtile_sb, 0.125)
            ident_sb = singles.tile([P, P], F32, tag="ident")
            make_identity(nc, ident_sb)
            # keep the gpsimd queue warm so the first real broadcast
            # dispatches without a wakeup stall
            gp_warm = singles.tile([P, 8], F32, tag="gp_warm")
            nc.gpsimd.partition_broadcast(gp_warm, ident_sb[0:1, 0:8])

            # warm up the PE while the qw DMA is in flight; keepalives
            # are N=512 so the att region's PE-busy fraction stays high
            warm_ps = ps_misc.tile([P, 512], F32, tag="ps_misc")

            def warm_mm(n=1, cols=B):
                for _ in range(n):
                    nc.tensor.matmul(warm_ps[:, 0:cols],
                                     lhsT=wtile_sb[:, 0:P],
                                     rhs=wtile_sb[:, 0:cols],
                                     start=True, stop=True)

            warm_mm(WARMUP_MMS)

            # ---- routing: att = qh@Wh + qh@Wl + ql@Wh + bk ----
            # three N=64 matmuls per chunk accumulate into ONE [128,64]
            # PSUM region (no fold op needed; a DVE tensor_tensor cannot
            # read two PSUM operands). Borrow the out banks.
            att_ps = [ps_out.tile([P, N_SUB], F32, tag=f"out{bt}",
                                  name=f"att{bt}") for bt in range(BT)]

            def qh_c(c, bt):
                o = QW_QH + c * B + bt * P
                return qw_sb[:, o:o + P]

            def ql_c(c, bt):
                o = QW_QL + c * B + bt * P
                return qw_sb[:, o:o + P]

            for c in range(QC):
                for bt in range(BT):
                    wh = qw_sb[:, c * P:c * P + N_SUB]
                    wl = qw_sb[:, c * P + N_SUB:(c + 1) * P]
                    nc.tensor.matmul(att_ps[bt], lhsT=qh_c(c, bt), rhs=wh,
                                     start=(c == 0), stop=False)
                    nc.tensor.matmul(att_ps[bt], lhsT=qh_c(c, bt), rhs=wl,
                                     start=False, stop=False)
                if c % 3 == 2:
                    warm_mm(cols=512)

            aT8_sb = singles.tile([E_LOC, B], FP16, tag="aT8")
            recip_bt = [singles.tile([P, 1], F32, tag=f"recip{bt}",
                                     name=f"recip{bt}") for bt in range(BT)]
            flat_sb = singles.tile([1, E_LOC * B], FP16, tag="flat")

            def emit_softmax():
                hp = tc.high_priority()
                hp.__enter__()
                for bt in range(BT):
                    # ---- top-8 + softmax (rows = batch) ----
                    m8 = work.tile([P, 8], F32, tag="m8")
                    nc.vector.max(out=m8, in_=att_ps[bt])
                    e_top = work.tile([P, N_SUB], F32, tag="e_top")
                    nc.scalar.activation(e_top, att_ps[bt],
                                         mybir.ActivationFunctionType.Exp)
                    # e = (att >= t8) * e_top, denom = row-sum(e). e stays
                    # UNNORMALIZED: 1/denom is applied at the output copy.
                    e = work.tile([P, N_SUB], F32, tag="e")
                    denom = work.tile([P, 1], F32, tag="denom")
                    nc.vector.scalar_tensor_tensor(
                        e, att_ps[bt], m8[:, 7:8], e_top,
                        op0=mybir.AluOpType.is_ge, op1=mybir.AluOpType.mult,
                        accum_out=denom)
                    nc.vector.reciprocal(recip_bt[bt], denom)
                    # transpose the local-expert block -> [8, P], cast to
                    # fp16 on the PSUM->SBUF copy
                    aT8_ps = ps_misc.tile([E_LOC, P], F32, tag="ps_misc")
                    nc.tensor.transpose(aT8_ps, e[:, 0:E_LOC], ident_sb)
                    nc.vector.tensor_copy(aT8_sb[:, bt * P:(bt + 1) * P],
                                          aT8_ps)
                hp.__exit__(None, None, None)
                warm_mm()

            # ---- expert loop, software-pipelined PIPE deep on the PE.
            # Experts 0-1 issue their MM1s BEFORE the ql half of the
            # attention (their xv piece lands before ql); the softmax
            # chain is emitted before any hs multiply so the DVE queue
            # cannot deadlock on the broadcast dependency.
            out_ps = [ps_out.tile([P, OUT_F], F32, tag=f"out{bt}",
                                  name=f"out_ps{bt}")
                      for bt in range(BT)]
            h_sbs = {}
            hs_tiles = {}

            def mm1_block(j):
                h_ps = ps_h.tile([P, B], F32, tag="h")
                for c in range(KC):
                    v0 = xv_sb[:, XV_V0 + j * KC * SUB_F + c * SUB_F:
                               XV_V0 + j * KC * SUB_F + (c + 1) * SUB_F]
                    nc.tensor.matmul(h_ps, lhsT=v0,
                                     rhs=xv_sb[:, c * B:(c + 1) * B],
                                     start=(c == 0), stop=(c == KC - 1))
                # h: PSUM fp32 -> SBUF fp16 on the ACT engine
                h_sb = work.tile([P, B], FP16, tag="h_sb")
                nc.scalar.activation(h_sb, h_ps,
                                     mybir.ActivationFunctionType.Copy)
                h_sbs[j] = h_sb

            def mm2(j):
                hs_sb = hs_tiles.pop(j)
                for bt in range(BT):
                    for nh in range(2):
                        nc.tensor.matmul(
                            out_ps[bt][:, nh * 512:(nh + 1) * 512],
                            lhsT=hs_sb[:, bt * P:(bt + 1) * P],
                            rhs=v1t_sb[j // 4][:, j % 4,
                                               nh * 512:(nh + 1) * 512],
                            start=(j == 0), stop=(j == E_LOC - 1),
                        )

            # ql correction + bias close the attention accumulation
            for c in range(QC):
                for bt in range(BT):
                    nc.tensor.matmul(
                        att_ps[bt], lhsT=ql_c(c, bt),
                        rhs=qw_sb[:, c * P:c * P + N_SUB],
                        start=False, stop=False)
            for bt in range(BT):
                nc.tensor.matmul(
                    att_ps[bt], lhsT=ones_sb,
                    rhs=qw_sb[0:1, QW_BIAS:QW_BIAS + N_SUB],
                    start=False, stop=True)

            emit_softmax()
            nc.sync.dma_start(flat_sb, aT8_sb)

            for j in range(E_LOC):
                mm1_block(j)
                # broadcast a[:, expert j] across partitions (fp16)
                abc_sb = work.tile([P, B], FP16, tag="abc")
                src = aT8_sb[0:1, :] if j == 0 \
                    else flat_sb[:, j * B:(j + 1) * B]
                nc.gpsimd.partition_broadcast(abc_sb, src)
                # hs = h * a  (fp16 x fp16 -> fp16, 2x-mode DVE)
                hs_sb = work.tile([P, B], FP16, tag="hs")
                nc.vector.tensor_tensor(hs_sb, h_sbs.pop(j), abc_sb,
                                        mybir.AluOpType.mult)
                hs_tiles[j] = hs_sb
                if j >= PIPE:
                    mm2(j - PIPE)
            for j in range(E_LOC - PIPE, E_LOC):
                mm2(j)

            # ---- write out: regions close in order (bt0nh0, bt0nh1,
            # bt1nh0, bt1nh1). DVE copies r1/r2/r4, ACT r3 (so the two
            # last-closing regions copy in parallel); one store per
            # batch tile, split across both rings ----
            for bt in range(BT):
                for nh in range(2):
                    o_sb = work.tile([P, 512], FP16, tag="o_sb")
                    srcr = out_ps[bt][:, nh * 512:(nh + 1) * 512]
                    if bt == 1 and nh == 0:
                        nc.scalar.activation(
                            o_sb, srcr, mybir.ActivationFunctionType.Copy,
                            scale=recip_bt[bt])
                    else:
                        nc.vector.tensor_scalar_mul(o_sb, srcr,
                                                    recip_bt[bt])
                    eng = nc.sync if bt == 0 else nc.scalar
                    eng.dma_start(
                        out_d[bt * P:(bt + 1) * P,
                              nh * 512:(nh + 1) * 512], o_sb)

    nc.compile()
    return nc


_CACHE = {}


def _get_nc(mode: str):
    if mode not in _CACHE:
        _CACHE[mode] = _build(mode)
    return _CACHE[mode]


def _pmajor(aT):
    """[D, N] (D = C*128, row-major) -> [128, C, N] partition-major."""
    d, n = aT.shape
    return np.ascontiguousarray(
        aT.reshape(d // P, P, n).transpose(1, 0, 2))


def _prep_in_maps(x, q, Wk, bk, V0, V1, mode: str):
    import ml_dtypes
    f16 = np.float16

    # hi/lo fp16 split of q (shared across cores)
    qT = _pmajor(q.T.astype(np.float32))                  # [128, QC, B] f32
    qh = qT.astype(f16)
    ql = (qT - qh.astype(np.float32)).astype(f16)
    qh = qh.reshape(P, QC * B)
    ql = ql.reshape(P, QC * B)
    xT = _pmajor(x.T).astype(f16).reshape(P, KC * B)      # [128, KC*B]
    # all-expert partition-major views, shared across the per-core loop:
    v0pm = V0.transpose(0, 2, 1).reshape(
        N_SUB, KC, P, SUB_F).transpose(0, 2, 1, 3)        # [S, P, KC, SUB]
    v1pm = V1.transpose(0, 2, 1)
    in_maps = []
    for c in range(N_CORES):
        rot = np.roll(np.arange(N_SUB), -E_LOC * c)
        wk_pm = _pmajor(Wk[rot].T.astype(np.float32))     # [128, QC, S] f32
        wh = wk_pm.astype(f16)
        wl = (wk_pm - wh.astype(np.float32)).astype(f16)
        whl = np.concatenate([wh, wl], axis=2).reshape(P, QC * 2 * N_SUB)
        bias_blk = np.zeros((P, P), f16)
        bias_blk[:, 0:N_SUB] = bk[rot].astype(f16)
        qw = np.concatenate([whl, bias_blk, qh, ql], axis=1)
        assert qw.shape == (P, QW_COLS)
        base = E_LOC * c
        v0t = v0pm[base:base + E_LOC].transpose(1, 0, 2, 3).reshape(
            P, E_LOC * KC * SUB_F).astype(f16)
        xv = np.ascontiguousarray(np.concatenate([xT, v0t], axis=1))
        assert xv.shape == (P, XV_COLS)
        v1t = v1pm[base:base + E_LOC].reshape(
            2, 4, SUB_F, OUT_F).transpose(0, 2, 1, 3).astype(
            ml_dtypes.float8_e3m4, order="C")
        in_maps.append({"qw": qw, "xv": xv, "v1t": v1t})
    return in_maps


def run(inputs: dict, mode: str = MOE_DTYPE, trace: bool = False):
    """Run the distributed kernel; returns (out [B, OUT_F] fp32, results)."""
    nc = _get_nc(mode)
    in_maps = _prep_in_maps(**inputs, mode=mode)
    res = bass_utils.run_bass_kernel_spmd(
        nc, in_maps, core_ids=list(range(N_CORES)), trace=trace,
    )
    out = np.zeros((B, OUT_F), np.float32)
    for c in range(N_CORES):
        out += np.asarray(res.results[c]["out_p"], dtype=np.float32)
    return out, res


def kernel(x, q, Wk, bk, V0, V1):
    x = np.asarray(x, np.float32)
    q = np.asarray(q, np.float32)
    Wk = np.asarray(Wk, np.float32)
    bk = np.asarray(bk, np.float32)
    V0 = np.asarray(V0, np.float32)
    V1 = np.asarray(V1, np.float32)
    out, _ = run(dict(x=x, q=q, Wk=Wk, bk=bk, V0=V0, V1=V1))
    return out
